# revision 39
# baseline (speedup 1.0000x reference)
"""KoLeo loss kernel for Trainium2 (8 NeuronCores).

Computes -mean(log(||x_i - x_{nn(i)} + eps||)) where x = row-normalized
student_output and nn(i) is the nearest neighbor by max inner product
(diagonal excluded).

For unit vectors ||x_i - x_j||^2 = 2 - 2*<x_i, x_j>, so only the per-row max
off-diagonal inner product m_i is needed. Each core handles a 2048-row block:
it receives the full matrix rotated so its own rows sit at local rows 0..2047
(SPMD-uniform diagonal masking).

Phase 1 builds XT8 = x-hat^T quantized to fp8e4 scaled by 16, as [128, 2, N]
(d-half major) for DoubleRow matmuls. It runs as two software-pipelined
passes: pass A computes row sum-of-squares (ACT batched squares + DVE 3D
reduce) and the rsqrt chain for all batches; pass B normalizes to bf16 on the
pool engine (broadcast tensor_tensor multiply), transposes each 128x128 tile
on the PE (bf16 is_transpose), and quantizes to fp8 inside the PSUM->SBUF
drain copies (ACT activation + DVE tensor_copy casts).

Phase 2 computes the [2048, 16384] dots block with DoubleRow fp8 matmuls
(K=256 in one matmul via the [128, 2, N] pair layout, dots scaled x256) into
[128, 1024] PSUM chunks (4 buffers deep, giving the PE a runway that hides
drain and semaphore latency), group-major so each chunk only depends on one
phase-1 batch and the two phases overlap. Chunks are drained alternately by
DVE (plain PSUM reduce_max; group-0 chunks get the diagonal masked by a
tensor_add first) and ACT (log-sum-exp: exp(dots - LSE_BIAS) accumulated per
row; host takes log and maxes it with the direct partials). The engine
pattern alternates in pairs so every PSUM chunk buffer alternates drain
engines. GPSIMD has no PSUM port and its small elementwise ops run far below
roofline, so it only does the bulk bf16 normalize multiplies.
tensor_tensor_reduce is avoided entirely: it wedges real TRN2 hardware.

The final log-mean runs on host from the tiny [128,*] outputs.
"""

import os

import numpy as np

import concourse.bass as bass
import concourse.mybir as mybir
import concourse.tile as tile
from concourse import bacc
from concourse import bass_utils
from concourse.masks import make_identity

N = 16384
D = 256
NCORES = 8
ROWS = N // NCORES          # 2048 rows per core
ITILES = ROWS // 128        # 16 i-tiles per core
NT = N // 128               # 128 row-tiles of the full matrix
GW = 2048                   # j-group width (4 PSUM banks of fp32)
NGROUPS = N // GW           # 8 j-groups
NB = 16                     # row-tiles per batch (= 1 group)
HB = NB // 2                # half-batch staged contiguously
SCALE = 16.0                # fp8 quantization scale; dots come out x256
MASKVAL = -1024.0           # diagonal knock-out (scaled dots are in [-290, 290])
EPS = 1e-8

# groups drained by ACT as log-sum-exp; the rest are DVE plain reduce_max.
ACT_GROUPS = (1, 2, 5, 6)
LSE_BIAS = 140.8            # = beta*C with beta=256 (dots scaled x256), C=0.55

_CACHE = {}

_BISECT_BATCHES = int(os.environ.get("KOLEO_BATCHES", NT // NB))
_BISECT_ITILES = int(os.environ.get("KOLEO_ITILES", ITILES))
_BISECT_GROUPS = int(os.environ.get("KOLEO_GROUPS", NGROUPS))


def _build():
    f32 = mybir.dt.float32
    bf16 = mybir.dt.bfloat16
    f8 = mybir.dt.float8e4
    AF = mybir.ActivationFunctionType
    ALU = mybir.AluOpType
    DR = mybir.MatmulPerfMode.DoubleRow

    nc = bacc.Bacc("TRN2", target_bir_lowering=False, debug=False)
    x = nc.dram_tensor("x", [N, D], f32, kind="ExternalInput").ap()
    m_out = nc.dram_tensor("m_out", [128, ITILES], f32, kind="ExternalOutput").ap()
    s_out = nc.dram_tensor(
        "s_out", [128, ITILES * NGROUPS * 2], f32, kind="ExternalOutput"
    ).ap()

    with tile.TileContext(nc) as tc:
        with (
            tc.tile_pool(name="singles", bufs=1) as singles,
            tc.tile_pool(name="s_stage", bufs=13) as s_stage,
            tc.tile_pool(name="small", bufs=6) as small,
            tc.tile_pool(name="sq_scr", bufs=3) as sq_scr,
            tc.tile_pool(name="qq", bufs=4) as q_pool,
            tc.tile_pool(name="xt", bufs=1) as xt_pool,
            tc.tile_pool(name="cp_scr", bufs=3) as cp_scr,
            tc.tile_pool(name="mp_pool", bufs=3) as mp_pool,
        ):
            identb = singles.tile([128, 128], bf16, tag="identb")
            make_identity(nc, identb[:])

            # Diagonal knock-out mask: MASKVAL on the diagonal of a 128x128 block.
            mneg = singles.tile([128, 128], f32, tag="mneg")
            nc.gpsimd.memset(mneg[:], 0.0)
            nc.gpsimd.affine_select(
                out=mneg[:],
                in_=mneg[:],
                compare_op=ALU.not_equal,
                fill=MASKVAL,
                base=0,
                pattern=[[-1, 128]],
                channel_multiplier=1,
            )

            # sum of squares per row, laid out [128, row-tile]
            ss = singles.tile([128, NT], f32, tag="ss")
            # per-row max accumulator, [128, i-tile]
            m_sb = singles.tile([128, ITILES], f32, tag="m_sb")
            # per-row LSE partial sums, [128, i-tile * act-group]
            s_sb = singles.tile([128, ITILES * NGROUPS * 2], f32, tag="s_sb")
            lse_bias = singles.tile([128, 1], f32, tag="lse_bias")
            nc.vector.memset(lse_bias[:], -LSE_BIAS)
            # XT8: transposed, normalized, fp8-quantized matrix.
            # Free layout [2, N]: d-half k at [:, k, :]; DoubleRow reads the
            # (k, col) pair dims directly.
            xt8 = xt_pool.tile([128, 2, N], f8, tag="xt8", name="xt8")
            if (_BISECT_BATCHES, _BISECT_ITILES, _BISECT_GROUPS) != (
                NT // NB, ITILES, NGROUPS
            ):
                nc.vector.memset(m_sb[:], 0.0)
                nc.vector.memset(s_sb[:], 0.0)
                nc.vector.memset(ss[:], 1.0)
                nc.gpsimd.memset(xt8[:, 0, :], 0.0)
                nc.gpsimd.memset(xt8[:, 1, :], 0.0)

            # ---- Phase 1: row norms, transpose+normalize, quantize ----
            # Two passes so the per-batch norm chain (sqrt/recip/newton) never
            # gates the quant+transpose pipeline: pass A computes all row
            # norms; pass B (scheduled data-driven, overlapping pass A and
            # phase 2) normalizes, transposes and quantizes.
            s_tiles = {}
            r16_all = singles.tile([128, NT], f32, tag="r16_all")
            with tc.tile_pool(name="tpsum", bufs=2, space="PSUM") as tpsum:
                for b in range(_BISECT_BATCHES):
                    t0 = b * NB
                    for h in range(2):
                        sb_t = s_stage.tile([128, HB, D], f32, tag="sbig")
                        for i in range(HB):
                            t = t0 + h * HB + i
                            nc.sync.dma_start(
                                out=sb_t[:, i, :],
                                in_=x[t * 128:(t + 1) * 128, :],
                            )
                        sq = sq_scr.tile([128, HB, D], bf16, tag="sqb")
                        nc.scalar.activation(sq[:], sb_t[:], AF.Square)
                        nc.vector.reduce_sum(
                            ss[:, t0 + h * HB:t0 + (h + 1) * HB],
                            sq[:], axis=mybir.AxisListType.X,
                        )
                        s_tiles[(b, h)] = sb_t

                    # batched r = rsqrt(ss) with two Newton steps
                    # (ACT Sqrt is low-precision; DVE reciprocal is accurate)
                    ssb = ss[:, t0:t0 + NB]
                    sq_b = small.tile([128, NB], f32, tag="sqb")
                    nc.scalar.activation(sq_b[:], ssb, AF.Sqrt)
                    r = small.tile([128, NB], f32, tag="r")
                    nc.vector.reciprocal(r[:], sq_b[:])
                    for _ in range(2):
                        t1 = small.tile([128, NB], f32, tag="t1")
                        nc.vector.tensor_mul(t1[:], r[:], r[:])
                        nc.vector.tensor_mul(t1[:], t1[:], ssb)
                        # t1 <- 1.5 - 0.5*t1
                        nc.scalar.activation(t1[:], t1[:], AF.Copy, scale=-0.5, bias=1.5)
                        r2 = small.tile([128, NB], f32, tag="r")
                        nc.vector.tensor_mul(r2[:], r[:], t1[:])
                        r = r2
                    nc.vector.tensor_scalar_mul(r16_all[:, t0:t0 + NB], r[:], SCALE)

                for b in range(_BISECT_BATCHES):
                    t0 = b * NB
                    # normalize to bf16 on the (otherwise idle) pool engine,
                    # then plain bf16 PE transposes
                    tp = [
                        tpsum.tile([128, NB * 128], bf16, tag=f"tp{k}", name=f"tp{k}")
                        for k in range(2)
                    ]
                    for h in range(2):
                        q = q_pool.tile([128, HB, D], bf16, tag="q")
                        nc.gpsimd.tensor_tensor(
                            q[:], s_tiles[(b, h)][:],
                            r16_all[:, t0 + h * HB:t0 + (h + 1) * HB]
                            .broadcast_to([128, HB, D]),
                            op=ALU.mult,
                        )
                        for i in range(HB):
                            w = h * HB + i
                            for k in range(2):
                                nc.tensor.transpose(
                                    tp[k][:, w * 128:(w + 1) * 128],
                                    q[:, i, k * 128:(k + 1) * 128],
                                    identb[:],
                                )
                    # drain + fp8 quantize (GpSimd has no PSUM port)
                    nc.scalar.activation(
                        xt8[:, 0, b * GW:(b + 1) * GW], tp[0][:], AF.Copy
                    )
                    nc.vector.tensor_copy(
                        xt8[:, 1, b * GW:(b + 1) * GW], tp[1][:]
                    )

            # ---- Phase 2: DoubleRow dots + row max / LSE ----
            # Group-major so chunk (t, g) only needs batch g (+ batch 0 for
            # the stationary): phase 2 pipelines under phase 1. Drains
            # alternate DVE (plain reduce_max) / ACT (exp accumulate) in
            # pairs so each PSUM buffer alternates drain engines.
            mdp = singles.tile([128, ITILES * NGROUPS * 2], f32, tag="mdp")
            nc.vector.memset(mdp[:], MASKVAL)
            nc.vector.memset(s_sb[:], 0.0)
            HW2 = GW // 2
            with (
                tc.tile_pool(name="dpsA", bufs=1, space="PSUM") as dpsA,
                tc.tile_pool(name="dpsD", bufs=2, space="PSUM") as dpsD,
            ):
                idx = 0
                for g in range(_BISECT_GROUPS):
                    for t in range(_BISECT_ITILES):
                        lhsT = xt8[:, :, t * 128:(t + 1) * 128]  # [128, 2, 128]
                        if idx % 2 == 0:
                            # ACT chunk: full 2048 cols, one exp+accumulate
                            pg = dpsA.tile([128, GW], f32, tag="pga")
                            for s4 in range(4):
                                j0 = g * GW + s4 * 512
                                nc.tensor.matmul(
                                    pg[:, s4 * 512:(s4 + 1) * 512],
                                    lhsT, xt8[:, :, j0:j0 + 512],
                                    start=True, stop=True, perf_mode=DR,
                                )
                            if g == 0:
                                db = 128 * t
                                nc.vector.tensor_add(
                                    pg[:, db:db + 128], pg[:, db:db + 128],
                                    mneg[:],
                                )
                            slot = (t * NGROUPS + g) * 2
                            sc = cp_scr.tile([128, GW], bf16, tag="cp")
                            nc.scalar.activation(
                                sc[:], pg[:], AF.Exp, bias=lse_bias[:],
                                accum_out=s_sb[:, slot:slot + 1],
                            )
                        else:
                            # DVE chunk: two 1024-col sub-chunks
                            for sc4 in range(2):
                                pg = dpsD.tile([128, HW2], f32, tag="pgd")
                                for s4 in range(2):
                                    j0 = g * GW + sc4 * HW2 + s4 * 512
                                    nc.tensor.matmul(
                                        pg[:, s4 * 512:(s4 + 1) * 512],
                                        lhsT, xt8[:, :, j0:j0 + 512],
                                        start=True, stop=True, perf_mode=DR,
                                    )
                                if g == 0 and (128 * t) // HW2 == sc4:
                                    db = 128 * t - sc4 * HW2
                                    nc.vector.tensor_add(
                                        pg[:, db:db + 128],
                                        pg[:, db:db + 128], mneg[:],
                                    )
                                slot = (t * NGROUPS + g) * 2 + sc4
                                nc.vector.reduce_max(
                                    mdp[:, slot:slot + 1],
                                    pg[:], axis=mybir.AxisListType.X,
                                )
                        idx += 1
                for t in range(ITILES):
                    nc.vector.reduce_max(
                        m_sb[:, t:t + 1],
                        mdp[:, t * NGROUPS * 2:(t + 1) * NGROUPS * 2],
                        axis=mybir.AxisListType.X,
                    )

            nc.sync.dma_start(out=m_out, in_=m_sb[:])
            nc.sync.dma_start(out=s_out, in_=s_sb[:])

    nc.compile()
    return nc


def _get_nc():
    if "nc" not in _CACHE:
        _CACHE["nc"] = _build()
    return _CACHE["nc"]


def kernel(student_output: np.ndarray) -> np.ndarray:
    s = np.ascontiguousarray(np.asarray(student_output, dtype=np.float32))
    assert s.shape == (N, D)

    nc = _get_nc()
    in_maps = [
        {"x": np.ascontiguousarray(np.roll(s, -c * ROWS, axis=0))}
        for c in range(NCORES)
    ]
    kwargs = {}
    if os.environ.get("KOLEO_TRACE"):
        kwargs = {"trace": True, "tmpdir": os.environ.get("KOLEO_TRACE_DIR") or None}
    res = bass_utils.run_bass_kernel_spmd(
        nc, in_maps, core_ids=list(range(NCORES)), **kwargs
    )
    _CACHE["last_results"] = res

    nls = NGROUPS * 2
    m = np.concatenate(
        [res.results[c]["m_out"].T.reshape(ROWS) for c in range(NCORES)]
    )  # [N] per-row max over the direct groups (scaled by 256)
    ssum = np.concatenate(
        [res.results[c]["s_out"].T.reshape(ITILES, nls, 128)
         .transpose(0, 2, 1).reshape(ROWS, nls)
         for c in range(NCORES)]
    )  # [N, nls] per-row LSE partial sums for the ACT groups

    with np.errstate(divide="ignore"):
        m_lse = np.log(ssum.astype(np.float64)).max(axis=1) + LSE_BIAS
    mm = np.maximum(m.astype(np.float64), m_lse) / (SCALE * SCALE)
    d2 = np.maximum(2.0 - 2.0 * mm, 0.0)
    loss = -np.mean(np.log(np.sqrt(d2) + EPS))
    return np.array(loss, dtype=np.float32)
